# revision 5
# baseline (speedup 1.0000x reference)
"""Trainium2 Bass kernel for nn_PinyinGRUEmbeddings.

Math: x = emb_eff[tokens] ([B,T,8], emb row 0 zeroed), two stacked GRU
layers (torch gate order r,z,n), output = layer-2 final hidden [B,8] fp32.

Strategy (pure data parallel over 8 cores, B=131072 -> 16384/core):
  - Host: embed tokens (tiny 27x8 table gather) and pack activations into
    tile layout [T, NBLK, 128, NJ] where partition p = 8*g + h packs 16
    independent sequence groups of the 8-wide hidden dim, and the free dim
    j indexes NJ sequences per group. One [128, NJ] tile = 16*NJ sequences.
  - Device: all four gate matmuls (input-side and hidden-side, both
    layers) are block-diagonal [128,128] x [128,NJ] PE matmuls
    accumulating in PSUM (gate-pure banks R, Z', Ni, Nh). All biases fold
    into ACT per-partition bias / scalar_tensor_tensor scalars.
    z' trick: weights/biases of the z gate are negated so
    sigmoid gives z' = 1 - z and h' = h + z' * (n - h).
  - Engine split per cell: PE 6 matmuls; ACT sigmoid r, sigmoid z',
    tanh n; DVE stt u=(Nh+b)*r, v=Ni+u, d=n-h; GPSIMD e=z'*d, h'=h+e.
"""

import numpy as np

import concourse.bacc as bacc
import concourse.tile as tile
from concourse import mybir
from concourse.bass_utils import run_bass_kernel_spmd

FP32 = mybir.dt.float32
AF = mybir.ActivationFunctionType
OP = mybir.AluOpType

H = 8
VOCAB = 27
N_CORES = 8
B_FULL = 131072
T_FULL = 24
G = 16          # sequence groups per tile (16 * 8 = 128 partitions)
NJ_FULL = 512   # sequences per group per tile (one PSUM bank of fp32)
NBLK_FULL = 2   # tiles per core: 2 * 16 * 512 = 16384 seqs/core


def build_program(T=T_FULL, NBLK=NBLK_FULL, NJ=NJ_FULL):
    nc = bacc.Bacc(None, target_bir_lowering=False)
    x_d = nc.declare_dram_parameter("x", [T, NBLK, 128, NJ], FP32, isOutput=False)
    w_d = nc.declare_dram_parameter("w", [128, 12 * 128], FP32, isOutput=False)
    b_d = nc.declare_dram_parameter("b", [128, 8], FP32, isOutput=False)
    o_d = nc.declare_dram_parameter("out", [NBLK, 128, NJ], FP32, isOutput=True)

    with tile.TileContext(nc) as tc:
        with (
            tc.tile_pool(name="wpool", bufs=1) as wpool,
            tc.tile_pool(name="hpool", bufs=1) as hpool,
            tc.tile_pool(name="xpool", bufs=4) as xpool,
            tc.tile_pool(name="tpool", bufs=3) as tpool,
            tc.tile_pool(name="psum", bufs=2, space="PSUM") as psum,
        ):
            wt = wpool.tile([128, 12 * 128], FP32, name="wt")
            bt = wpool.tile([128, 8], FP32, name="bt")
            nc.sync.dma_start(wt[:], w_d[:])
            nc.sync.dma_start(bt[:], b_d[:])

            def W(i):
                return wt[:, 128 * i : 128 * (i + 1)]

            def Bc(i):
                return bt[:, i : i + 1]

            h = {}
            for L in (1, 2):
                for blk in range(NBLK):
                    for par in (0, 1):
                        h[(L, blk, par)] = hpool.tile(
                            [128, NJ], FP32, name=f"h{L}_{blk}_{par}"
                        )

            def cell(L, t, xin, Hp, Hn):
                off = 0 if L == 1 else 6
                bo = 0 if L == 1 else 4
                first = t == 0
                R = psum.tile([128, NJ], FP32, name="Rp")
                Z = psum.tile([128, NJ], FP32, name="Zp")
                NI = psum.tile([128, NJ], FP32, name="NIp")
                nc.tensor.matmul(R[:], W(off + 0), xin[:], start=True, stop=first)
                nc.tensor.matmul(Z[:], W(off + 1), xin[:], start=True, stop=first)
                nc.tensor.matmul(NI[:], W(off + 2), xin[:], start=True, stop=True)
                if not first:
                    NH = psum.tile([128, NJ], FP32, name="NHp")
                    nc.tensor.matmul(R[:], W(off + 3), Hp[:], start=False, stop=True)
                    nc.tensor.matmul(Z[:], W(off + 4), Hp[:], start=False, stop=True)
                    nc.tensor.matmul(NH[:], W(off + 5), Hp[:], start=True, stop=True)
                r = tpool.tile([128, NJ], FP32, name="r")
                z = tpool.tile([128, NJ], FP32, name="z")
                nc.scalar.activation(r[:], R[:], AF.Sigmoid, bias=Bc(bo + 0))
                nc.scalar.activation(z[:], Z[:], AF.Sigmoid, bias=Bc(bo + 1))
                u = tpool.tile([128, NJ], FP32, name="u")
                if first:
                    nc.vector.tensor_scalar_mul(u[:], r[:], Bc(bo + 2))
                else:
                    nc.vector.scalar_tensor_tensor(
                        u[:], NH[:], Bc(bo + 2), r[:], op0=OP.add, op1=OP.mult
                    )
                nc.vector.tensor_tensor(NI[:], NI[:], u[:], op=OP.add)
                n = tpool.tile([128, NJ], FP32, name="n")
                nc.scalar.activation(n[:], NI[:], AF.Tanh, bias=Bc(bo + 3))
                if first:
                    nc.gpsimd.tensor_tensor(Hn[:], z[:], n[:], op=OP.mult)
                else:
                    d = tpool.tile([128, NJ], FP32, name="d")
                    nc.vector.tensor_sub(d[:], n[:], Hp[:])
                    e = tpool.tile([128, NJ], FP32, name="e")
                    nc.gpsimd.tensor_tensor(e[:], z[:], d[:], op=OP.mult)
                    nc.gpsimd.tensor_tensor(Hn[:], Hp[:], e[:], op=OP.add)

            # Software-pipelined wavefront: layer 1 runs one timestep ahead of
            # layer 2 — cell(1, t+1) and cell(2, t) are independent, giving the
            # engines 2*NBLK concurrent chains to overlap.
            for blk in range(NBLK):
                xt = xpool.tile([128, NJ], FP32, name="xt")
                nc.sync.dma_start(xt[:], x_d[0, blk])
                cell(1, 0, xt, h[(1, blk, 0)], h[(1, blk, 1)])
            for t in range(T):
                for blk in range(NBLK):
                    if t + 1 < T:
                        xt = xpool.tile([128, NJ], FP32, name="xt")
                        nc.sync.dma_start(xt[:], x_d[t + 1, blk])
                        cell(
                            1, t + 1, xt,
                            h[(1, blk, (t + 1) % 2)], h[(1, blk, t % 2)],
                        )
                    cell(
                        2, t, h[(1, blk, (t + 1) % 2)],
                        h[(2, blk, t % 2)], h[(2, blk, (t + 1) % 2)],
                    )

            for blk in range(NBLK):
                nc.sync.dma_start(o_d[blk], h[(2, blk, T % 2)][:])

    return nc


def _block_diag_lhsT(Wg, negate=False):
    # Wg: [8, 8] gate block (rows = output h, cols = input h).
    # lhsT[k, m] = Wg[m, k]; block-diag over 16 groups.
    A = Wg.T.astype(np.float32)
    if negate:
        A = -A
    return np.kron(np.eye(G, dtype=np.float32), A)


def pack_weights(w_ih1, w_hh1, b_ih1, b_hh1, w_ih2, w_hh2, b_ih2, b_hh2):
    mats = []
    for Wfull in (w_ih1, w_hh1, w_ih2, w_hh2):
        Wfull = np.asarray(Wfull, dtype=np.float32)
        for gate in range(3):
            blkm = Wfull[8 * gate : 8 * gate + 8, :]
            mats.append(_block_diag_lhsT(blkm, negate=(gate == 1)))
    wblob = np.ascontiguousarray(np.concatenate(mats, axis=1))  # [128, 1536]

    b_ih1 = np.asarray(b_ih1, np.float32)
    b_hh1 = np.asarray(b_hh1, np.float32)
    b_ih2 = np.asarray(b_ih2, np.float32)
    b_hh2 = np.asarray(b_hh2, np.float32)

    def t16(v):
        return np.tile(v.astype(np.float32), G)

    cols = [
        t16(b_ih1[0:8] + b_hh1[0:8]),        # sigmoid bias r, L1
        t16(-(b_ih1[8:16] + b_hh1[8:16])),   # sigmoid bias z' (negated), L1
        t16(b_hh1[16:24]),                   # stt scalar (b_hh n), L1
        t16(b_ih1[16:24]),                   # tanh bias (b_ih n), L1
        t16(b_ih2[0:8] + b_hh2[0:8]),
        t16(-(b_ih2[8:16] + b_hh2[8:16])),
        t16(b_hh2[16:24]),
        t16(b_ih2[16:24]),
    ]
    bblob = np.ascontiguousarray(np.stack(cols, axis=1))  # [128, 8]
    return wblob, bblob


def pack_x(tokens, emb, n_cores=N_CORES, T=T_FULL, NBLK=NBLK_FULL, NJ=NJ_FULL):
    # tokens [B, T] int, emb [27, 8]; returns [n_cores, T, NBLK, 128, NJ] fp32
    tokens = np.asarray(tokens).astype(np.int64)
    emb_eff = np.asarray(emb, dtype=np.float32).copy()
    emb_eff[0] = 0.0
    x_full = emb_eff[tokens]  # [B, T, 8]
    B = tokens.shape[0]
    assert B == n_cores * NBLK * G * NJ and tokens.shape[1] == T
    xp = x_full.reshape(n_cores, NBLK, G, NJ, T, H)
    xp = xp.transpose(0, 4, 1, 2, 5, 3)  # [c, t, blk, g, h, j]
    return np.ascontiguousarray(xp.reshape(n_cores, T, NBLK, 128, NJ))


def unpack_out(outs, n_cores=N_CORES, NBLK=NBLK_FULL, NJ=NJ_FULL):
    # outs: list of [NBLK, 128, NJ] per core -> [B, 8]
    o = np.stack([np.asarray(x) for x in outs])  # [c, blk, 128, NJ]
    o = o.reshape(n_cores, NBLK, G, H, NJ).transpose(0, 1, 2, 4, 3)
    return np.ascontiguousarray(o.reshape(n_cores * NBLK * G * NJ, H))


def run(inputs, trace=False, **spmd_kwargs):
    xp = pack_x(inputs["inputs"], inputs["emb"])
    wblob, bblob = pack_weights(
        inputs["w_ih1"], inputs["w_hh1"], inputs["b_ih1"], inputs["b_hh1"],
        inputs["w_ih2"], inputs["w_hh2"], inputs["b_ih2"], inputs["b_hh2"],
    )
    nc = build_program()
    nc.finalize()
    in_maps = [
        {"x": np.ascontiguousarray(xp[c]), "w": wblob, "b": bblob}
        for c in range(N_CORES)
    ]
    res = run_bass_kernel_spmd(
        nc, in_maps, list(range(N_CORES)), trace=trace, **spmd_kwargs
    )
    out = unpack_out([res.results[c]["out"] for c in range(N_CORES)])
    return out, res


def kernel(**inputs) -> np.ndarray:
    out, _ = run(inputs)
    return out


# revision 16
# speedup vs baseline: 1.6333x; 1.6333x over previous
"""Trainium2 Bass kernel for nn_PinyinGRUEmbeddings.

Math: x = emb_eff[tokens] ([B,T,8], emb row 0 zeroed), two stacked GRU
layers (torch gate order r,z,n), output = layer-2 final hidden [B,8] fp32.

Strategy (pure data parallel over 8 cores, B=131072 -> 16384/core):
  - Host: embed tokens (tiny 27x8 table gather) and pack activations into
    tile layout [T, NBLK, 128, NJ] where partition p = 8*g + h packs 16
    independent sequence groups of the 8-wide hidden dim, and the free dim
    j indexes NJ sequences per group. One [128, NJ] tile = 16*NJ sequences.
  - Device: all four gate matmuls (input-side and hidden-side, both
    layers) are block-diagonal [128,128] x [128,NJ] PE matmuls
    accumulating in PSUM (gate-pure banks R, Z', Ni, Nh). All biases fold
    into ACT per-partition bias / scalar_tensor_tensor scalars.
    z' trick: weights/biases of the z gate are negated so
    sigmoid gives z' = 1 - z and h' = h + z' * (n - h).
  - Engine split per cell: PE 6 matmuls; ACT sigmoid r, sigmoid z',
    tanh n; DVE stt u=(Nh+b)*r, v=Ni+u, d=n-h; GPSIMD e=z'*d, h'=h+e.
"""

import numpy as np

import concourse.bacc as bacc
import concourse.tile as tile
from concourse import mybir
from concourse.bass_utils import run_bass_kernel_spmd

FP32 = mybir.dt.float32
FP16 = mybir.dt.float16
AF = mybir.ActivationFunctionType
OP = mybir.AluOpType

H = 8
VOCAB = 27
N_CORES = 8
B_FULL = 131072
T_FULL = 24
G = 16          # sequence groups per tile (16 * 8 = 128 partitions)
NJ_FULL = 512   # sequences per group per tile (one PSUM bank of fp32)
NBLK_FULL = 2   # tiles per core: 2 * 16 * 512 = 16384 seqs/core


def build_program(T=T_FULL, NBLK=NBLK_FULL, NJ=NJ_FULL):
    nc = bacc.Bacc(None, target_bir_lowering=False)
    x_d = nc.declare_dram_parameter("x", [T, NBLK, 128, NJ], FP16, isOutput=False)
    w_d = nc.declare_dram_parameter("w", [128, 12 * 128], FP16, isOutput=False)
    b_d = nc.declare_dram_parameter("b", [128, 8], FP32, isOutput=False)
    o_d = nc.declare_dram_parameter("out", [NBLK, 128, NJ], FP16, isOutput=True)

    with tile.TileContext(nc) as tc:
        with (
            tc.tile_pool(name="wpool", bufs=1) as wpool,
            tc.tile_pool(name="hpool", bufs=1) as hpool,
            tc.tile_pool(name="xpool", bufs=4) as xpool,
            tc.tile_pool(name="tpool", bufs=3) as tpool,
            tc.tile_pool(name="psum", bufs=2, space="PSUM") as psum,
        ):
            wt = wpool.tile([128, 12 * 128], FP16, name="wt")
            bt = wpool.tile([128, 8], FP32, name="bt")
            nc.sync.dma_start(wt[:], w_d[:])
            nc.sync.dma_start(bt[:], b_d[:])

            def W(i):
                return wt[:, 128 * i : 128 * (i + 1)]

            def Bc(i):
                return bt[:, i : i + 1]

            h = {}
            for L in (1, 2):
                for blk in range(NBLK):
                    for par in (0, 1):
                        h[(L, blk, par)] = hpool.tile(
                            [128, NJ], FP16, name=f"h{L}_{blk}_{par}"
                        )

            def cell(L, t, xin, Hp, Hn):
                off = 0 if L == 1 else 6
                bo = 0 if L == 1 else 4
                first = t == 0
                R = psum.tile([128, NJ], FP32, name="Rp")
                Z = psum.tile([128, NJ], FP32, name="Zp")
                NI = psum.tile([128, NJ], FP32, name="NIp")
                nc.tensor.matmul(R[:], W(off + 0), xin[:], start=True, stop=first)
                nc.tensor.matmul(Z[:], W(off + 1), xin[:], start=True, stop=first)
                nc.tensor.matmul(NI[:], W(off + 2), xin[:], start=True, stop=True)
                if not first:
                    NH = psum.tile([128, NJ], FP32, name="NHp")
                    nc.tensor.matmul(R[:], W(off + 3), Hp[:], start=False, stop=True)
                    nc.tensor.matmul(Z[:], W(off + 4), Hp[:], start=False, stop=True)
                    nc.tensor.matmul(NH[:], W(off + 5), Hp[:], start=True, stop=True)
                r = tpool.tile([128, NJ], FP16, name="r")
                z = tpool.tile([128, NJ], FP16, name="z")
                nc.scalar.activation(r[:], R[:], AF.Sigmoid, bias=Bc(bo + 0))
                nc.scalar.activation(z[:], Z[:], AF.Sigmoid, bias=Bc(bo + 1))
                u = tpool.tile([128, NJ], FP32, name="u")
                if first:
                    nc.vector.tensor_scalar_mul(u[:], r[:], Bc(bo + 2))
                else:
                    nc.vector.scalar_tensor_tensor(
                        u[:], NH[:], Bc(bo + 2), r[:], op0=OP.add, op1=OP.mult
                    )
                nc.vector.tensor_tensor(NI[:], NI[:], u[:], op=OP.add)
                n = tpool.tile([128, NJ], FP16, name="n")
                nc.scalar.activation(n[:], NI[:], AF.Tanh, bias=Bc(bo + 3))
                if first:
                    nc.gpsimd.tensor_tensor(Hn[:], z[:], n[:], op=OP.mult)
                else:
                    d = tpool.tile([128, NJ], FP16, name="d")
                    nc.vector.tensor_sub(d[:], n[:], Hp[:])
                    e = tpool.tile([128, NJ], FP16, name="e")
                    nc.gpsimd.tensor_tensor(e[:], z[:], d[:], op=OP.mult)
                    nc.gpsimd.tensor_tensor(Hn[:], Hp[:], e[:], op=OP.add)

            # Software-pipelined wavefront: layer 1 runs one timestep ahead of
            # layer 2 — cell(1, t+1) and cell(2, t) are independent, giving the
            # engines 2*NBLK concurrent chains to overlap.
            for blk in range(NBLK):
                xt = xpool.tile([128, NJ], FP16, name="xt")
                nc.sync.dma_start(xt[:], x_d[0, blk])
                cell(1, 0, xt, h[(1, blk, 0)], h[(1, blk, 1)])
            for t in range(T):
                for blk in range(NBLK):
                    if t + 1 < T:
                        xt = xpool.tile([128, NJ], FP16, name="xt")
                        nc.sync.dma_start(xt[:], x_d[t + 1, blk])
                        cell(
                            1, t + 1, xt,
                            h[(1, blk, (t + 1) % 2)], h[(1, blk, t % 2)],
                        )
                    cell(
                        2, t, h[(1, blk, (t + 1) % 2)],
                        h[(2, blk, t % 2)], h[(2, blk, (t + 1) % 2)],
                    )

            for blk in range(NBLK):
                nc.sync.dma_start(o_d[blk], h[(2, blk, T % 2)][:])

    return nc


def _block_diag_lhsT(Wg, negate=False):
    # Wg: [8, 8] gate block (rows = output h, cols = input h).
    # lhsT[k, m] = Wg[m, k]; block-diag over 16 groups.
    A = Wg.T.astype(np.float32)
    if negate:
        A = -A
    return np.kron(np.eye(G, dtype=np.float32), A)


def pack_weights(w_ih1, w_hh1, b_ih1, b_hh1, w_ih2, w_hh2, b_ih2, b_hh2):
    mats = []
    for Wfull in (w_ih1, w_hh1, w_ih2, w_hh2):
        Wfull = np.asarray(Wfull, dtype=np.float32)
        for gate in range(3):
            blkm = Wfull[8 * gate : 8 * gate + 8, :]
            mats.append(_block_diag_lhsT(blkm, negate=(gate == 1)))
    wblob = np.ascontiguousarray(
        np.concatenate(mats, axis=1).astype(np.float16)
    )  # [128, 1536]

    b_ih1 = np.asarray(b_ih1, np.float32)
    b_hh1 = np.asarray(b_hh1, np.float32)
    b_ih2 = np.asarray(b_ih2, np.float32)
    b_hh2 = np.asarray(b_hh2, np.float32)

    def t16(v):
        return np.tile(v.astype(np.float32), G)

    cols = [
        t16(b_ih1[0:8] + b_hh1[0:8]),        # sigmoid bias r, L1
        t16(-(b_ih1[8:16] + b_hh1[8:16])),   # sigmoid bias z' (negated), L1
        t16(b_hh1[16:24]),                   # stt scalar (b_hh n), L1
        t16(b_ih1[16:24]),                   # tanh bias (b_ih n), L1
        t16(b_ih2[0:8] + b_hh2[0:8]),
        t16(-(b_ih2[8:16] + b_hh2[8:16])),
        t16(b_hh2[16:24]),
        t16(b_ih2[16:24]),
    ]
    bblob = np.ascontiguousarray(np.stack(cols, axis=1))  # [128, 8]
    return wblob, bblob


def pack_x(tokens, emb, n_cores=N_CORES, T=T_FULL, NBLK=NBLK_FULL, NJ=NJ_FULL):
    # tokens [B, T] int, emb [27, 8]; returns [n_cores, T, NBLK, 128, NJ] fp32
    tokens = np.asarray(tokens).astype(np.int64)
    emb_eff = np.asarray(emb, dtype=np.float32).copy()
    emb_eff[0] = 0.0
    x_full = emb_eff[tokens]  # [B, T, 8]
    B = tokens.shape[0]
    assert B == n_cores * NBLK * G * NJ and tokens.shape[1] == T
    xp = x_full.reshape(n_cores, NBLK, G, NJ, T, H)
    xp = xp.transpose(0, 4, 1, 2, 5, 3)  # [c, t, blk, g, h, j]
    return np.ascontiguousarray(
        xp.reshape(n_cores, T, NBLK, 128, NJ).astype(np.float16)
    )


def unpack_out(outs, n_cores=N_CORES, NBLK=NBLK_FULL, NJ=NJ_FULL):
    # outs: list of [NBLK, 128, NJ] per core -> [B, 8]
    o = np.stack([np.asarray(x) for x in outs]).astype(np.float32)
    o = o.reshape(n_cores, NBLK, G, H, NJ).transpose(0, 1, 2, 4, 3)
    return np.ascontiguousarray(o.reshape(n_cores * NBLK * G * NJ, H))


def run(inputs, trace=False, **spmd_kwargs):
    xp = pack_x(inputs["inputs"], inputs["emb"])
    wblob, bblob = pack_weights(
        inputs["w_ih1"], inputs["w_hh1"], inputs["b_ih1"], inputs["b_hh1"],
        inputs["w_ih2"], inputs["w_hh2"], inputs["b_ih2"], inputs["b_hh2"],
    )
    nc = build_program()
    nc.finalize()
    in_maps = [
        {"x": np.ascontiguousarray(xp[c]), "w": wblob, "b": bblob}
        for c in range(N_CORES)
    ]
    res = run_bass_kernel_spmd(
        nc, in_maps, list(range(N_CORES)), trace=trace, **spmd_kwargs
    )
    out = unpack_out([res.results[c]["out"] for c in range(N_CORES)])
    return out, res


def kernel(**inputs) -> np.ndarray:
    out, _ = run(inputs)
    return out
